# revision 1
# baseline (speedup 1.0000x reference)
"""Trainium2 Bass kernel for nn_ButterflyFilter.

The reference chain (pad -> butterfly FFT -> ramp filter in bit-reversed
order -> butterfly IFFT -> Re[:512]) is linear in x, so it is one real
512x512 operator W = Re(A)[:512, :512] with A circulant. W is an exactly
symmetric Toeplitz matrix W[o, i] = g[o - i] with g the FBP ramp kernel
(g[0] = 1/2, g[odd d] = -2/(pi d)^2, g[even d] = 0), which decays like
1/d^2: a 64-wide staircase band changes the result by ~1.6e-4 relative;
with bf16 operands and output store the total is ~2.6e-3 (measured),
7x under the 2e-2 gate.

Banded + Toeplitz => each 128-row output chunk needs TWO input chunks on
a 64-shifted grid:
  out[128o : 128o+128] = Ga @ c_o + Gb @ c_{o+1},
  c_j = x rows [128j - 64, 128j + 64)   (zero-padded at the ends)
with the same two 128x128 stationaries for every o: 8 matmuls per
(b, c) tile, 16 per core (2 tiles/core, 8 cores), 64 KiB of operator.

Schedule facts this implementation is built around (from NTFF traces):
  - The DMA fabric is ~270 GB/s per core AGGREGATE across queues, so
    queues are specialized: Sync carries the 5 input pieces in exact PE
    consumption order; Scalar's queue carries outputs. Competing input
    streams starve the PE mid-stream.
  - A dma_start costs ~0.6-0.7 us of descriptor-gen on the issuing
    engine: inputs are fused into 5 pieces (64K + 256K/384K per tile)
    with 1-3 KiB partition lines.
  - Concurrently in-flight DMAs must not share a semaphore (their 16
    completion increments interleave out of order): one per piece.
  - PSUM->SBUF bf16 drains: DVE tensor_copy signals with then_inc
    directly (proven safe); ACT activation-copies are kept OFF the
    critical tail and signal via an explicit pipeline drain.
  - ~6 warm-up matmuls on garbage SBUF bridge program start to the
    first piece's arrival so the HAM clock ramp (1.2 -> 2.4 GHz after
    ~3.4 us of sustained PE activity) completes before the real stream;
    any PE idle gap resets the ramp credit.
"""

import os
import sys
import types
from contextlib import ExitStack

import numpy as np

import concourse.bass as bass
import concourse.mybir as mybir
from concourse.bass_utils import run_bass_kernel_spmd


def _ensure_axon_hooks():
    # concourse.bass_utils imports antenv.axon_hooks on the trace path; some
    # images lack that module. Provide a no-op holder so a BASS_TRACE env set
    # by the caller can't crash the run.
    try:
        import antenv.axon_hooks  # noqa: F401
    except Exception:
        m = types.ModuleType("antenv.axon_hooks")
        m._h = None
        m.set_axon_ntff_profile_hook = lambda h: setattr(m, "_h", h)
        m.get_axon_ntff_profile_hook = lambda: m._h
        sys.modules["antenv.axon_hooks"] = m


_ensure_axon_hooks()

N_CORES = 8
S = 512          # row length and angle count (moving dim)
NF = 1024        # padded length inside the reference
P = 128
H = 64           # chunk-grid shift
OC = 4           # output row chunks per tile
BC_PER_CORE = 2
N_WARM = int(os.environ.get("BUTTERFLY_NWARM", "21"))

last_exec_time_ns = None
last_results = None


def _butterfly_np(tw, x, increasing):
    B, n = x.shape
    m = tw.shape[0]
    order = range(m) if increasing else range(m - 1, -1, -1)
    for idx in order:
        s = 1 << idx
        t = tw[idx].reshape(n // (2 * s), s, 2, 2)
        xr = x.reshape(B, n // (2 * s), 2, s)
        x = np.einsum('gjik,bgkj->bgij', t, xr).reshape(B, n)
    return x


def _compose_w(twiddle_fft, twiddle_ifft, fourier_filter_br):
    """Fold twiddles+filter into the dense operator W[o, i] (512x512 f64)."""
    tw_fft = np.asarray(twiddle_fft, dtype=np.float64)
    tw_ifft = np.asarray(twiddle_ifft, dtype=np.float64)
    filt = np.asarray(fourier_filter_br, dtype=np.float64)
    tf = tw_fft[0, ..., 0] + 1j * tw_fft[0, ..., 1]
    ti = tw_ifft[0, ..., 0] + 1j * tw_ifft[0, ..., 1]
    X = np.eye(NF, dtype=np.complex128)
    X = _butterfly_np(tf, X, increasing=False)
    X = X * filt[None, :]
    X = _butterfly_np(ti, X, increasing=True)
    return np.real(X[:S, :S]).T.copy()


def _band_stationaries(W):
    """lhsT operands: lhsT_a[i', o'] = g[o'-i'+64], lhsT_b = g[o'-i'-64]."""
    g = W[:, 0]  # g[|d|]; W is symmetric Toeplitz to ~3e-8
    D = np.arange(P)[None, :] - np.arange(P)[:, None]  # D[i', o'] = o' - i'
    return g[np.abs(D + H)], g[np.abs(D - H)]


def _build_nc():
    bf16 = mybir.dt.bfloat16
    f32 = mybir.dt.float32

    nc = bass.Bass()
    # Input pieces in Sync-queue (= PE consumption) order, packed for fat
    # partition lines (4 KiB lines move ~257 GB/s vs ~190 at 2 KiB). The
    # operator piece goes alone first (64 KiB) so the stream starts early;
    # the c4 chunks ride a later piece (first needed by matmul #8).
    #   p0 = (128, 256)  [Ga | Gb]
    #   p1 = (128, 2048) [c0|c1|c2|c3] tile0     (4 KiB lines)
    #   pc4 = (128, 1024) [c4_t0 | c4_t1]
    #   p2 = (128, 1024) [c0|c1] tile1, p3 = (128, 1024) [c2|c3] tile1
    p0 = nc.declare_dram_parameter("p0", [P, 2 * P], bf16, isOutput=False)
    p1 = nc.declare_dram_parameter("p1", [P, 4 * S], bf16, isOutput=False)
    p2 = nc.declare_dram_parameter("p2", [P, 4 * S], bf16, isOutput=False)
    p3 = nc.declare_dram_parameter("p3", [P, 2 * S], bf16, isOutput=False)
    out0 = nc.declare_dram_parameter("out0", [P, OC * S], bf16, isOutput=True)
    out1 = nc.declare_dram_parameter("out1", [P, OC * S], bf16, isOutput=True)

    with ExitStack() as ctx:
        w_sb = ctx.enter_context(nc.sbuf_tensor("w_sb", [P, 2 * P], bf16))
        p1_sb = ctx.enter_context(nc.sbuf_tensor("p1_sb", [P, 4 * S], bf16))
        p2_sb = ctx.enter_context(nc.sbuf_tensor("p2_sb", [P, 4 * S], bf16))
        p3_sb = ctx.enter_context(nc.sbuf_tensor("p3_sb", [P, 2 * S], bf16))
        warm_sb = ctx.enter_context(nc.sbuf_tensor("warm_sb", [P, P + S], bf16))
        o_sb = [
            ctx.enter_context(nc.sbuf_tensor(f"o_sb{t}", [P, OC * S], bf16))
            for t in range(BC_PER_CORE)
        ]
        accs = [
            ctx.enter_context(nc.psum_tensor(f"acc{g}", [P, S], f32))
            for g in range(BC_PER_CORE * OC)
        ]
        s_i = [ctx.enter_context(nc.semaphore(f"s_i{j}")) for j in range(4)]
        s_pe = ctx.enter_context(nc.semaphore("s_pe"))
        s_cl = ctx.enter_context(nc.semaphore("s_cl"))   # DVE copies
        s_cr = ctx.enter_context(nc.semaphore("s_cr"))   # ACT copies
        s_out = ctx.enter_context(nc.semaphore("s_out"))
        block = ctx.enter_context(nc.Block())

        ga = w_sb[:, 0:P]
        gb = w_sb[:, P:2 * P]
        cs = [
            [
                p1_sb[:, bass.ts(j, S)] for j in range(4)
            ] + [p2_sb[:, bass.ts(0, S)]],
            [
                p2_sb[:, bass.ts(2, S)], p2_sb[:, bass.ts(3, S)],
                p3_sb[:, bass.ts(0, S)], p3_sb[:, bass.ts(1, S)],
                p2_sb[:, bass.ts(1, S)],
            ],
        ]

        @block.sync
        def _(sync):
            sync.dma_start(w_sb[:], p0[:]).then_inc(s_i[0], 16)
            sync.dma_start(p1_sb[:], p1[:]).then_inc(s_i[1], 16)
            sync.dma_start(p2_sb[:], p2[:]).then_inc(s_i[2], 16)
            sync.dma_start(p3_sb[:], p3[:]).then_inc(s_i[3], 16)
            # tile1 whole output (4 KiB lines move ~355 GB/s) once all of
            # tile1 is drained: DVE copies g4,g6,g7 (#3..#5) and ACT g5.
            sync.wait_ge(s_cl, 5)
            sync.wait_ge(s_cr, 3)
            sync.dma_start(out1[:], o_sb[1][:]).then_inc(s_out, 16)
            sync.wait_ge(s_out, 2 * 16)

        @block.tensor
        def _(tensor):
            # Warm-ups bridge program start to the first piece's arrival.
            # They must be full-K: the HAM activity monitor weighs how much
            # of the array is busy, and low-K warm-ups fail to earn the
            # 2.4 GHz grant (measured: K=4 left the whole stream at 1.2).
            for _ in range(N_WARM):
                nc.tensor.matmul(
                    accs[-1][:, :2 * P], warm_sb[:, :P], warm_sb[:, P:P + 2 * P],
                    start=True, stop=True,
                )
            tensor.wait_ge(s_i[0], 16)
            for t in range(BC_PER_CORE):
                a = OC * t
                c = cs[t]

                def mm(g, w_ap, c_ap, start, stop):
                    m = nc.tensor.matmul(
                        accs[g][:], w_ap, c_ap, start=start, stop=stop
                    )
                    if stop:
                        m.then_inc(s_pe, 1)

                # Ga/Gb alternated so a group closes every 2nd matmul — the
                # copy engines start draining as early as possible.
                if t == 0:
                    tensor.wait_ge(s_i[1], 16)
                    mm(a + 0, ga, c[0], True, False)
                    mm(a + 0, gb, c[1], False, True)
                    mm(a + 1, ga, c[1], True, False)
                    mm(a + 1, gb, c[2], False, True)
                    mm(a + 2, ga, c[2], True, False)
                    mm(a + 2, gb, c[3], False, True)
                    mm(a + 3, ga, c[3], True, False)
                    tensor.wait_ge(s_i[2], 16)
                    mm(a + 3, gb, c[4], False, True)
                else:
                    mm(a + 0, ga, c[0], True, False)
                    mm(a + 0, gb, c[1], False, True)
                    mm(a + 1, ga, c[1], True, False)
                    tensor.wait_ge(s_i[3], 16)
                    mm(a + 1, gb, c[2], False, True)
                    mm(a + 2, ga, c[2], True, False)
                    mm(a + 2, gb, c[3], False, True)
                    mm(a + 3, ga, c[3], True, False)
                    mm(a + 3, gb, c[4], False, True)

        @block.vector
        def _(vector):
            # DVE drains g0, g2 and the tail-critical g4, g6, g7; then_inc
            # rides the copy itself (v3.1-proven safe for DVE).
            for g in (0, 2, 4, 6, 7):
                t, o = divmod(g, OC)
                vector.wait_ge(s_pe, g + 1)
                nc.vector.tensor_copy(
                    o_sb[t][:, bass.ts(o, S)], accs[g][:]
                ).then_inc(s_cl, 1)

        @block.scalar
        def _(scalar):
            # ACT drains g1, g3, g5 back-to-back, signals once via a single
            # pipeline drain, then issues tile0's output on its queue.
            for g, (t, o) in ((1, (0, 1)), (3, (0, 3)), (5, (1, 1))):
                scalar.wait_ge(s_pe, g + 1)
                nc.scalar.copy(o_sb[t][:, bass.ts(o, S)], accs[g][:])
            scalar.drain().then_inc(s_cr, 3)
            scalar.wait_ge(s_cl, 2)
            scalar.dma_start(out0[:], o_sb[0][:]).then_inc(s_out, 16)

    return nc


def kernel(x, twiddle_fft, twiddle_ifft, fourier_filter_br):
    global last_exec_time_ns, last_results
    import ml_dtypes

    bf16 = ml_dtypes.bfloat16
    x = np.asarray(x, dtype=np.float32)
    b, c, s_len, a = x.shape
    assert (b, c, s_len, a) == (8, 2, S, S)

    W = _compose_w(twiddle_fft, twiddle_ifft, fourier_filter_br)
    la, lb = _band_stationaries(W)
    w_piece = np.ascontiguousarray(
        np.concatenate([la, lb], axis=1).astype(bf16)
    )
    x16 = x.reshape(b * c, S, S)
    zpad = np.zeros((H, S), dtype=bf16)

    in_maps = []
    for core in range(N_CORES):
        cks = []
        for t in range(BC_PER_CORE):
            xb = x16[BC_PER_CORE * core + t].astype(bf16)
            cks.append(
                [
                    np.concatenate([zpad, xb[0:H]], axis=0),
                    xb[H:H + P],
                    xb[H + P:H + 2 * P],
                    xb[H + 2 * P:H + 3 * P],
                    np.concatenate([xb[H + 3 * P:], zpad], axis=0),
                ]
            )
        cat = lambda parts: np.ascontiguousarray(np.concatenate(parts, axis=1))
        in_maps.append(
            {
                "p0": np.ascontiguousarray(w_piece),
                "p1": cat(cks[0][0:4]),
                "p2": cat([cks[0][4], cks[1][4]] + cks[1][0:2]),
                "p3": cat(cks[1][2:4]),
            }
        )
    nc = _build_nc()
    trace = os.environ.get("BUTTERFLY_TRACE") == "1"
    res = run_bass_kernel_spmd(nc, in_maps, core_ids=list(range(N_CORES)), trace=trace)
    last_exec_time_ns = res.exec_time_ns
    last_results = res

    # outN[p, 512*o + a] = proj row 128*o + p of tile 2*core + N.
    q = np.empty((b * c, S, S), dtype=np.float32)
    for k in range(N_CORES):
        for t, name in enumerate(("out0", "out1")):
            y = np.asarray(res.results[k][name]).reshape(P, OC, S)
            q[BC_PER_CORE * k + t] = (
                y.transpose(1, 0, 2).reshape(S, S).astype(np.float32)
            )
    # q[bc, o, a] = proj.T[o, bc*512 + a]; reference output is
    # proj.T.reshape(b, c, s, a) — a pure reinterpret of the (512, 8192) buffer.
    out = q.transpose(1, 0, 2).reshape(S, b * c * a).reshape(b, c, s_len, a)
    return np.ascontiguousarray(out).astype(np.float32)



# revision 2
# speedup vs baseline: 1.0689x; 1.0689x over previous
"""Trainium2 Bass kernel for nn_ButterflyFilter — v7.

Schedule derived from v1-v6 traces (see git of this file's history):
  - exec window = first framework MEMSET .. end of compiler epilogue
    (~7.8 us, starts after the LAST engine's work) => minimize last
    real-work time.
  - HBM reads cap ~270-310 GB/s, writes ~390-400 GB/s, and they OVERLAP
    additively (v6 probe measured 420+ GB/s combined) => stream output
    pieces as soon as each group pair drains.
  - One DMA engine sometimes runs slow (run-to-run roulette), making a
    piece's 16th completion increment lag its data by up to ~2 us; the
    lag is cumulative across the stream => split x into 4 x 256 KiB
    pieces so the FIRST piece's semaphore lands early and matmuls trail
    the stream.
  - HAM duty-cycles the clock (3.4-6.8 us full-rate grant starting
    ~3.4-5.7 us after sustained PE activity; idle gaps reset the
    credit; clamp halves matmul rate). Warm-up matmuls from block start
    bridge to the first piece and position the grant over the real
    matmuls (v1-proven; gating them later backfired in v5).
  - GpSimd DMA queue has ~2 us startup latency; SP and ACT dispatch in
    ~650 ns => all output DMAs ride SP's queue (idle after inputs).
  - ACT's activation table is preloaded at block start (saves 1.3 us).
  - Every dynamic DMA needs a semaphore; concurrent DMAs need distinct
    ones.

Band structure (unchanged from v1): out = Re(IFFT.filter.FFT)[:512] is
a symmetric Toeplitz operator, truncated to a 64-wide staircase band =>
out[128o:128(o+1)] = Ga @ c_o + Gb @ c_{o+1} on a 64-shifted chunk
grid; edge chunks use K=64 matmuls against partition-aligned slices
(no zero padding is ever DMA'd).
"""

import os
import sys
import types
from contextlib import ExitStack

import numpy as np

import concourse.bass as bass
import concourse.mybir as mybir
from concourse.bass_utils import run_bass_kernel_spmd


def _ensure_axon_hooks():
    try:
        import antenv.axon_hooks  # noqa: F401
    except Exception:
        m = types.ModuleType("antenv.axon_hooks")
        m._h = None
        m.set_axon_ntff_profile_hook = lambda h: setattr(m, "_h", h)
        m.get_axon_ntff_profile_hook = lambda: m._h
        sys.modules["antenv.axon_hooks"] = m


_ensure_axon_hooks()

N_CORES = 8
S = 512
NF = 1024
P = 128
H = 64
OC = 4
BC_PER_CORE = 2
N_WARM = int(os.environ.get("BUTTERFLY_NWARM", "17"))

last_exec_time_ns = None
last_results = None


def _butterfly_np(tw, x, increasing):
    B, n = x.shape
    m = tw.shape[0]
    order = range(m) if increasing else range(m - 1, -1, -1)
    for idx in order:
        s = 1 << idx
        t = tw[idx].reshape(n // (2 * s), s, 2, 2)
        xr = x.reshape(B, n // (2 * s), 2, s)
        x = np.einsum('gjik,bgkj->bgij', t, xr).reshape(B, n)
    return x


def _compose_w(twiddle_fft, twiddle_ifft, fourier_filter_br):
    tw_fft = np.asarray(twiddle_fft, dtype=np.float64)
    tw_ifft = np.asarray(twiddle_ifft, dtype=np.float64)
    filt = np.asarray(fourier_filter_br, dtype=np.float64)
    tf = tw_fft[0, ..., 0] + 1j * tw_fft[0, ..., 1]
    ti = tw_ifft[0, ..., 0] + 1j * tw_ifft[0, ..., 1]
    X = np.eye(NF, dtype=np.complex128)
    X = _butterfly_np(tf, X, increasing=False)
    X = X * filt[None, :]
    X = _butterfly_np(ti, X, increasing=True)
    return np.real(X[:S, :S]).T.copy()


def _band_stationaries(W):
    g = W[:, 0]
    D = np.arange(P)[None, :] - np.arange(P)[:, None]
    return g[np.abs(D + H)], g[np.abs(D - H)]


def _build_nc():
    bf16 = mybir.dt.bfloat16
    f32 = mybir.dt.float32

    nc = bass.Bass()
    pw = nc.declare_dram_parameter("pw", [P, 3 * P], bf16, isOutput=False)
    # x pieces: per tile, a = [c0s | c1], b = [c2 | c3]; c0s partitions
    # 0:64 = x rows 0:64 (chunk 0 edge), 64:128 = x rows 448:512 (chunk 3).
    px = [
        nc.declare_dram_parameter(f"px{j}", [P, 2 * S], bf16, isOutput=False)
        for j in range(4)
    ]
    out0 = nc.declare_dram_parameter("out0", [P, OC * S], bf16, isOutput=True)
    out1 = nc.declare_dram_parameter("out1", [P, OC * S], bf16, isOutput=True)

    with ExitStack() as ctx:
        w_sb = ctx.enter_context(nc.sbuf_tensor("w_sb", [P, 3 * P], bf16))
        x_sb = [
            ctx.enter_context(nc.sbuf_tensor(f"x_sb{j}", [P, 2 * S], bf16))
            for j in range(4)
        ]
        warm_sb = ctx.enter_context(nc.sbuf_tensor("warm_sb", [P, 4 * P], bf16))
        o_sb = [
            ctx.enter_context(nc.sbuf_tensor(f"o_sb{t}", [P, OC * S], bf16))
            for t in range(BC_PER_CORE)
        ]
        accs = [
            ctx.enter_context(nc.psum_tensor(f"acc{g}", [P, S], f32))
            for g in range(BC_PER_CORE * OC)
        ]
        s_i = [ctx.enter_context(nc.semaphore(f"s_i{j}")) for j in range(5)]
        s_pe = ctx.enter_context(nc.semaphore("s_pe"))
        s_cl = ctx.enter_context(nc.semaphore("s_cl"))
        s_cr = ctx.enter_context(nc.semaphore("s_cr"))
        s_o = [ctx.enter_context(nc.semaphore(f"s_o{j}")) for j in range(4)]
        block = ctx.enter_context(nc.Block())

        ga = w_sb[:, 0:P]
        gb = w_sb[:, P:2 * P]
        ga_lo = w_sb[0:H, 2 * P:3 * P]
        gb_hi = w_sb[H:P, 2 * P:3 * P]

        @block.sync
        def _(sync):
            sync.dma_start(w_sb[:], pw[:]).then_inc(s_i[0], 16)
            for j in range(4):
                sync.dma_start(x_sb[j][:], px[j][:]).then_inc(s_i[j + 1], 16)
            # Output pieces: one per drained group pair, streamed while
            # later input pieces / matmuls are still in flight.
            outs = (out0[:, 0:2 * S], out0[:, 2 * S:4 * S],
                    out1[:, 0:2 * S], out1[:, 2 * S:4 * S])
            srcs = (o_sb[0][:, 0:2 * S], o_sb[0][:, 2 * S:4 * S],
                    o_sb[1][:, 0:2 * S], o_sb[1][:, 2 * S:4 * S])
            for j in range(4):
                sync.wait_ge(s_cl, j + 1)
                sync.wait_ge(s_cr, j + 1)
                sync.dma_start(outs[j], srcs[j]).then_inc(s_o[j], 16)
            for j in range(4):
                sync.wait_ge(s_o[j], 16)

        @block.tensor
        def _(tensor):
            for _ in range(N_WARM):
                nc.tensor.matmul(
                    accs[-1][:, :2 * P], warm_sb[:, :P], warm_sb[:, P:3 * P],
                    start=True, stop=True,
                )
            tensor.wait_ge(s_i[0], 16)

            def mm(g, w_ap, c_ap, start, stop):
                m = nc.tensor.matmul(
                    accs[g][:], w_ap, c_ap, start=start, stop=stop
                )
                if stop:
                    m.then_inc(s_pe, 1)

            for t in range(BC_PER_CORE):
                a = OC * t
                xa, xb_ = x_sb[2 * t], x_sb[2 * t + 1]
                c0 = xa[0:H, 0:S]
                c4 = xa[H:P, 0:S]
                c1 = xa[:, S:2 * S]
                c2 = xb_[:, 0:S]
                c3 = xb_[:, S:2 * S]
                tensor.wait_ge(s_i[2 * t + 1], 16)
                mm(a + 0, ga_lo, c0, True, False)
                mm(a + 0, gb, c1, False, True)
                mm(a + 1, ga, c1, True, False)
                tensor.wait_ge(s_i[2 * t + 2], 16)
                mm(a + 1, gb, c2, False, True)
                mm(a + 2, ga, c2, True, False)
                mm(a + 2, gb, c3, False, True)
                mm(a + 3, ga, c3, True, False)
                mm(a + 3, gb_hi, c4, False, True)

        @block.vector
        def _(vector):
            for g in (0, 2, 4, 6):
                t, o = divmod(g, OC)
                vector.wait_ge(s_pe, g + 1)
                nc.vector.tensor_copy(
                    o_sb[t][:, bass.ts(o, S)], accs[g][:]
                ).then_inc(s_cl, 1)

        @block.scalar
        def _(scalar):
            nc.scalar.copy(warm_sb[:, 3 * P:4 * P], warm_sb[:, 0:P])
            for g in (1, 3, 5, 7):
                t, o = divmod(g, OC)
                scalar.wait_ge(s_pe, g + 1)
                nc.scalar.copy(o_sb[t][:, bass.ts(o, S)], accs[g][:])
                scalar.drain().then_inc(s_cr, 1)

    return nc


def kernel(x, twiddle_fft, twiddle_ifft, fourier_filter_br):
    global last_exec_time_ns, last_results
    import ml_dtypes

    bf16 = ml_dtypes.bfloat16
    x = np.asarray(x, dtype=np.float32)
    b, c, s_len, a = x.shape
    assert (b, c, s_len, a) == (8, 2, S, S)

    W = _compose_w(twiddle_fft, twiddle_ifft, fourier_filter_br)
    la, lb = _band_stationaries(W)
    edge = np.concatenate([la[H:P, :], lb[0:H, :]], axis=0)
    w_piece = np.ascontiguousarray(
        np.concatenate([la, lb, edge], axis=1).astype(bf16)
    )
    x16 = x.reshape(b * c, S, S)

    in_maps = []
    for core in range(N_CORES):
        m = {"pw": w_piece}
        for t in range(BC_PER_CORE):
            xb = x16[BC_PER_CORE * core + t].astype(bf16)
            c0s = np.concatenate([xb[0:H], xb[S - H:S]], axis=0)
            m[f"px{2 * t}"] = np.ascontiguousarray(
                np.concatenate([c0s, xb[H:H + P]], axis=1))
            m[f"px{2 * t + 1}"] = np.ascontiguousarray(
                np.concatenate([xb[H + P:H + 2 * P], xb[H + 2 * P:H + 3 * P]],
                               axis=1))
        in_maps.append(m)
    nc = _build_nc()
    trace = os.environ.get("BUTTERFLY_TRACE") == "1"
    res = run_bass_kernel_spmd(nc, in_maps, core_ids=list(range(N_CORES)), trace=trace)
    last_exec_time_ns = res.exec_time_ns
    last_results = res

    q = np.empty((b * c, S, S), dtype=np.float32)
    for k in range(N_CORES):
        for t, name in enumerate(("out0", "out1")):
            y = np.asarray(res.results[k][name]).reshape(P, OC, S)
            q[BC_PER_CORE * k + t] = (
                y.transpose(1, 0, 2).reshape(S, S).astype(np.float32)
            )
    out = q.transpose(1, 0, 2).reshape(S, b * c * a).reshape(b, c, s_len, a)
    return np.ascontiguousarray(out).astype(np.float32)
